# revision 1
# baseline (speedup 1.0000x reference)
"""GAU (Gated Attention Unit, relu^2 attention) Trainium2 Bass kernel.

Problem shapes: x [4, 2048, 2048] f32; W_hidden [2048, 8192]; W_qk [2048, 128];
W_out [4096, 2048]; out = GAU(x) + x.

Sharding (8 cores): core = 2*batch + h, h in {0,1}.  Each pair of cores
handles one batch; within the pair the hidden dim (v cols 4096, gate cols
4096) is column-split in half (h picks cols [h*2048:(h+1)*2048] of v and of
gate, and rows [h*2048:(h+1)*2048] of W_out).  The qk projection (128 wide)
and the 2048x2048 attention matrix are replicated within the pair (~3% extra
flops).  Each core produces a partial output [2048, 2048] (its W_out-half
contribution) with the residual x pre-added on the rows it owns; a pairwise
ReduceScatter(add) then leaves each core with its final [1024, 2048] row
block, which the host concatenates.

Dataflow per core (all matmuls bf16 operands, fp32 PSUM accumulation; the
branch contribution is ~5e-4 of the output scale, so bf16 keeps overall
relative error ~1e-5):

  xT   [d, i]  (host-pretransposed bf16)
  qkT  [e, i] = silu(Wqk^T x^T + b_qk)        lhsT=Wqk[d,e],  rhs=xT[d,i]
  qT/kT = gamma*qkT + beta (per-partition affine)
  v    [j, c] = silu(x Whv + b_hv)            lhsT=xT[d,j],   rhs=Whv[d,c]
  gateT[c, i] = silu(Whg^T x^T + b_hg)        lhsT=Whg[d,c],  rhs=xT[d,i]   (spilled to DRAM)
  attnT[j, i] = relu(qkT^T qkT / seq)^2       lhsT=kT[e,j],   rhs=qT[e,i]
  ogT  [c, i] = (v^T attnT) * gateT           lhsT=v[j,c],    rhs=attnT[j,i]
  part [i, d] = ogT^T Wout_h (+b_out +x_own)  lhsT=ogT[c,i],  rhs=Wout[c,d]

No on-device transposes are needed: every contraction has both operands
naturally laid out with the contraction dim on partitions.  Free-dim biases
(b_hidden v-part, b_out) are added with K=1 rank-1 matmuls into PSUM;
partition-dim biases (b_qk, b_hidden gate-part) use the activation bias port.
"""

import numpy as np
import ml_dtypes
from contextlib import ExitStack

import concourse.bass as bass
import concourse.bacc as bacc
import concourse.mybir as mybir
import concourse.tile as tile
from concourse.bass_utils import run_bass_kernel_spmd

BF16 = mybir.dt.bfloat16
F32 = mybir.dt.float32
AF = mybir.ActivationFunctionType
ALU = mybir.AluOpType
P = 128


def build_gau_nc(seq=2048, dim=2048, hh=2048, n_cores=8,
                 IC=None, CC=None, DC=None, with_bhv=True):
    """Build the SPMD program.  hh = per-core hidden half width."""
    e = P  # qk dim
    nd = dim // P       # d-tiles (contraction tiles for x)
    njt = seq // P      # seq tiles (j)
    IC = IC or min(512, seq)  # i-chunk (moving free dim)
    n_ic = seq // IC
    CC = CC or min(512, hh)   # c-chunk for v
    n_cc = hh // CC
    nct = hh // P       # c-tiles
    DC = DC or min(512, dim)  # d-chunk for the output matmul
    n_dc = dim // DC
    n_it = IC // P      # i-tiles per i-chunk
    pairs = [[2 * g, 2 * g + 1] for g in range(n_cores // 2)]

    nc = bacc.Bacc("TRN2", target_bir_lowering=False, debug=False,
                   num_devices=n_cores)

    xT_d = nc.dram_tensor("xT", [dim, seq], BF16, kind="ExternalInput")
    whv_d = nc.dram_tensor("whv", [dim, hh], BF16, kind="ExternalInput")
    whg_d = nc.dram_tensor("whg", [dim, hh], BF16, kind="ExternalInput")
    wqk_d = nc.dram_tensor("wqk", [P, dim], BF16, kind="ExternalInput")
    wout_d = nc.dram_tensor("wout", [hh, dim], BF16, kind="ExternalInput")
    bqk_d = nc.dram_tensor("bqk", [e, 1], F32, kind="ExternalInput")
    gq_d = nc.dram_tensor("gq", [e, 1], F32, kind="ExternalInput")
    bq_d = nc.dram_tensor("bq", [e, 1], F32, kind="ExternalInput")
    gk_d = nc.dram_tensor("gk", [e, 1], F32, kind="ExternalInput")
    bk_d = nc.dram_tensor("bk", [e, 1], F32, kind="ExternalInput")
    bhv_d = nc.dram_tensor("bhv", [1, hh], BF16, kind="ExternalInput")
    bhgT_d = nc.dram_tensor("bhgT", [P, nct], F32, kind="ExternalInput")
    xres_d = nc.dram_tensor("xres", [seq // 2, dim], F32, kind="ExternalInput")
    out_d = nc.dram_tensor("out", [seq // 2, dim], F32, kind="ExternalOutput")

    with TileCtx(nc) as tc, ExitStack() as st:
        constp = st.enter_context(tc.tile_pool(name="const", bufs=1))
        psump = st.enter_context(tc.tile_pool(name="psum", bufs=8, space="PSUM"))
        dramp = st.enter_context(tc.tile_pool(name="dram", bufs=1, space="DRAM"))
        mainp = st.enter_context(tc.tile_pool(name="main", bufs=1))

        gtd = dramp.tile([hh, seq], BF16, tag="gtd", name="gtd")  # gateT spill
        # per-128-row-block reduce buffers: each block's ReduceScatter can
        # launch as soon as its partial rows are written (overlaps compute)
        # and the final (serial) reduce quantum is small
        pb = [dramp.tile([P, dim], BF16, tag=f"pb{k}", name=f"pb{k}")
              for k in range(njt)]
        rb = [dramp.tile([P // 2, dim], BF16, tag=f"rb{k}", name=f"rb{k}")
              for k in range(njt)]

        # ---- constants ----
        wqk_sb = constp.tile([P, nd * e], BF16, tag="wqk")
        nc.sync.dma_start(wqk_sb[:], wqk_d[:])
        bqk_sb = constp.tile([e, 1], F32, tag="bqk")
        nc.sync.dma_start(bqk_sb[:], bqk_d[:])
        gq_sb = constp.tile([e, 1], F32, tag="gq")
        nc.sync.dma_start(gq_sb[:], gq_d[:])
        bq_sb = constp.tile([e, 1], F32, tag="bq")
        nc.sync.dma_start(bq_sb[:], bq_d[:])
        gk_sb = constp.tile([e, 1], F32, tag="gk")
        nc.sync.dma_start(gk_sb[:], gk_d[:])
        bk_sb = constp.tile([e, 1], F32, tag="bk")
        nc.sync.dma_start(bk_sb[:], bk_d[:])
        bhgT_sb = constp.tile([P, nct], F32, tag="bhgT")
        nc.sync.dma_start(bhgT_sb[:], bhgT_d[:])
        bhv_sb = constp.tile([1, hh], BF16, tag="bhv")
        nc.sync.dma_start(bhv_sb[:], bhv_d[:])
        ones_sb = constp.tile([1, P], BF16, tag="ones")
        nc.vector.memset(ones_sb[:], 1.0)

        # tiny ReduceScatter to warm the collective stream while the PE is
        # busy with the projections -- the first real RS otherwise pays a
        # ~50us cold-start that stalls the pipeline
        warm_in = dramp.tile([2, 64], F32, tag="warm_in", name="warm_in")
        warm_out = dramp.tile([1, 64], F32, tag="warm_out", name="warm_out")
        warm_sb = constp.tile([2, 64], F32, tag="warm_sb")
        nc.vector.memset(warm_sb[:], 0.0)
        nc.gpsimd.dma_start(warm_in[:], warm_sb[:])
        nc.gpsimd.collective_compute("ReduceScatter", ALU.add,
                                     replica_groups=pairs,
                                     ins=[warm_in.opt()],
                                     outs=[warm_out.opt()])

        # persistent activations
        qT_sb = mainp.tile([e, seq], BF16, tag="qT", name="qT")
        kT_sb = mainp.tile([e, seq], BF16, tag="kT", name="kT")
        v_sb = [mainp.tile([P, hh], BF16, tag=f"v{jt}", name=f"v{jt}") for jt in range(njt)]

        with tc.tile_pool(name="ph1", bufs=1) as ph1p, \
             tc.tile_pool(name="wstream", bufs=1) as wsp:
            xT_sb = [ph1p.tile([P, seq], BF16, tag=f"xT{d}", name=f"xT{d}") for d in range(nd)]
            for half in range(2):
                cols = slice(half * (seq // 2), (half + 1) * (seq // 2))
                for d in range(nd):
                    nc.sync.dma_start(xT_sb[d][:, cols],
                                      xT_d[d * P:(d + 1) * P, cols])

            # ---- qk projection ----
            # silu(u) = u * sigmoid(u); the interp has no Silu LUT, so build
            # it from Sigmoid (ACT) + mult (DVE) everywhere.
            with tc.tile_pool(name="qkp", bufs=1) as qkp:
                qk_sb = qkp.tile([e, seq], F32, tag="qk", name="qk")
                for ic in range(n_ic):
                    ps = psump.tile([P, IC], F32, tag="ps", name="ps")
                    for d in range(nd):
                        nc.tensor.matmul(ps[:], wqk_sb[:, d * e:(d + 1) * e],
                                         xT_sb[d][:, ic * IC:(ic + 1) * IC],
                                         start=(d == 0), stop=(d == nd - 1))
                    sg = qkp.tile([P, IC], F32, tag="sg1", bufs=2, name="sg")
                    nc.scalar.activation(sg[:], ps[:], AF.Sigmoid,
                                         bias=bqk_sb[:])
                    u = qkp.tile([P, IC], F32, tag="u1", bufs=2, name="u")
                    nc.vector.tensor_scalar_add(u[:], ps[:], bqk_sb[:])
                    nc.vector.tensor_tensor(qk_sb[:, ic * IC:(ic + 1) * IC],
                                            u[:], sg[:], ALU.mult)
                nc.vector.tensor_scalar(qT_sb[:], qk_sb[:], gq_sb[:],
                                        bq_sb[:], ALU.mult, ALU.add)
                nc.vector.tensor_scalar(kT_sb[:], qk_sb[:], gk_sb[:],
                                        bk_sb[:], ALU.mult, ALU.add)

            # ---- hidden, v part: v[j, c] ----
            for cc in range(n_cc):
                wv = [wsp.tile([P, CC], BF16, tag=f"wv{d}", bufs=1, name=f"wv{d}")
                      for d in range(nd)]
                for d in range(nd):
                    nc.sync.dma_start(wv[d][:],
                                      whv_d[d * P:(d + 1) * P,
                                            cc * CC:(cc + 1) * CC])
                for jt in range(njt):
                    ps = psump.tile([P, CC], F32, tag="ps", name="ps")
                    for d in range(nd):
                        nc.tensor.matmul(ps[:], xT_sb[d][:, jt * P:(jt + 1) * P],
                                         wv[d][:], start=(d == 0),
                                         stop=(not with_bhv and d == nd - 1))
                    if with_bhv:
                        nc.tensor.matmul(ps[:], ones_sb[:],
                                         bhv_sb[:, cc * CC:(cc + 1) * CC],
                                         start=False, stop=True)
                    sg = wsp.tile([P, CC], F32, tag="sgv", bufs=2, name="sgv")
                    nc.scalar.activation(sg[:], ps[:], AF.Sigmoid)
                    nc.vector.tensor_tensor(v_sb[jt][:, cc * CC:(cc + 1) * CC],
                                            ps[:], sg[:], ALU.mult)

            # ---- hidden, gate part: gateT[c, i] -> DRAM ----
            for ct in range(nct):
                wg = [wsp.tile([P, P], BF16, tag=f"wg{d}", bufs=2, name=f"wg{d}")
                      for d in range(nd)]
                for d in range(nd):
                    nc.sync.dma_start(wg[d][:],
                                      whg_d[d * P:(d + 1) * P,
                                            ct * P:(ct + 1) * P])
                for ic in range(n_ic):
                    ps = psump.tile([P, IC], F32, tag="ps", name="ps")
                    for d in range(nd):
                        nc.tensor.matmul(ps[:], wg[d][:],
                                         xT_sb[d][:, ic * IC:(ic + 1) * IC],
                                         start=(d == 0), stop=(d == nd - 1))
                    sg = wsp.tile([P, IC], F32, tag="sgg", bufs=2, name="sgg")
                    nc.scalar.activation(sg[:], ps[:], AF.Sigmoid,
                                         bias=bhgT_sb[:, ct:ct + 1])
                    u = wsp.tile([P, IC], F32, tag="ug", bufs=2, name="ug")
                    nc.vector.tensor_scalar_add(u[:], ps[:],
                                                bhgT_sb[:, ct:ct + 1])
                    gstage = wsp.tile([P, IC], BF16, tag="gstage", bufs=2, name="gstage")
                    nc.vector.tensor_tensor(gstage[:], u[:], sg[:], ALU.mult)
                    nc.scalar.dma_start(gtd[ct * P:(ct + 1) * P,
                                            ic * IC:(ic + 1) * IC], gstage[:])

        # ---- attention + output, per i-chunk ----
        # chunk widths: full IC chunks, with the last split (IC-P, P) so the
        # final (serial) block reduce covers only P rows and earlier blocks'
        # reduces hide under the small chunk's compute
        widths = [IC] * n_ic
        with tc.tile_pool(name="ph2", bufs=1) as ph2p:
            at_sb = [ph2p.tile([P, IC], BF16, tag=f"at{jt}", name=f"at{jt}") for jt in range(njt)]
            og_sb = [ph2p.tile([P, IC], BF16, tag=f"og{ct}", name=f"og{ct}") for ct in range(nct)]
            cstart = 0
            for cw in widths:
                n_it_c = cw // P
                # attnT[j, chunk] = relu(sim/seq)^2, bf16
                for jt in range(njt):
                    ps = psump.tile([P, cw], F32, tag="ps", name="ps")
                    nc.tensor.matmul(ps[:], kT_sb[:, jt * P:(jt + 1) * P],
                                     qT_sb[:, cstart:cstart + cw],
                                     start=True, stop=True)
                    rstage = ph2p.tile([P, cw], F32, tag="rstage", bufs=4, name="rstage")
                    nc.scalar.activation(rstage[:], ps[:], AF.Relu,
                                         scale=1.0 / seq)
                    nc.vector.tensor_tensor(at_sb[jt][:, :cw], rstage[:],
                                            rstage[:], ALU.mult)
                # ogT[c, chunk] = (v^T @ attnT) * gateT
                for ct in range(nct):
                    ps = psump.tile([P, cw], F32, tag="ps", name="ps")
                    for jt in range(njt):
                        nc.tensor.matmul(ps[:], v_sb[jt][:, ct * P:(ct + 1) * P],
                                         at_sb[jt][:, :cw],
                                         start=(jt == 0), stop=(jt == njt - 1))
                    g = ph2p.tile([P, cw], BF16, tag="g", bufs=4, name="g")
                    nc.sync.dma_start(g[:], gtd[ct * P:(ct + 1) * P,
                                                cstart:cstart + cw])
                    nc.vector.tensor_tensor(og_sb[ct][:, :cw], ps[:], g[:],
                                            ALU.mult)
                # partial[chunk rows, :] = ogT^T @ Wout
                for dc in range(n_dc):
                    wo = [ph2p.tile([P, DC], BF16, tag=f"wo{ct}", bufs=2, name=f"wo{ct}")
                          for ct in range(nct)]
                    for ct in range(nct):
                        nc.sync.dma_start(wo[ct][:],
                                          wout_d[ct * P:(ct + 1) * P,
                                                 dc * DC:(dc + 1) * DC])
                    for it in range(n_it_c):
                        i_abs = cstart // P + it
                        ps = psump.tile([P, DC], F32, tag="ps", name="ps")
                        for ct in range(nct):
                            nc.tensor.matmul(ps[:],
                                             og_sb[ct][:, it * P:(it + 1) * P],
                                             wo[ct][:],
                                             start=(ct == 0),
                                             stop=(ct == nct - 1))
                        po = ph2p.tile([P, DC], BF16, tag="po", bufs=4, name="po")
                        nc.vector.tensor_copy(po[:], ps[:])
                        nc.scalar.dma_start(
                            pb[i_abs][:, dc * DC:(dc + 1) * DC], po[:])
                # pairwise reduce-scatter per 128-row block; core h of a pair
                # gets rows [h*64, (h+1)*64) of each block
                oh = P // 2
                for it in range(n_it_c):
                    k = cstart // P + it
                    nc.gpsimd.collective_compute("ReduceScatter", ALU.add,
                                                 replica_groups=pairs,
                                                 ins=[pb[k].opt()],
                                                 outs=[rb[k].opt()])
                    orow = k * oh
                    for dc in range(n_dc):
                        rd = ph2p.tile([oh, DC], BF16, tag="rd", bufs=4,
                                       name="rd")
                        nc.gpsimd.dma_start(rd[:],
                                            rb[k][:, dc * DC:(dc + 1) * DC])
                        xr = ph2p.tile([oh, DC], F32, tag="xr", bufs=4,
                                       name="xr")
                        nc.gpsimd.dma_start(xr[:],
                                            xres_d[orow:orow + oh,
                                                   dc * DC:(dc + 1) * DC])
                        fo = ph2p.tile([oh, DC], F32, tag="fo", bufs=4,
                                       name="fo")
                        nc.vector.tensor_tensor(fo[:], xr[:], rd[:], ALU.add)
                        nc.scalar.dma_start(out_d[orow:orow + oh,
                                                  dc * DC:(dc + 1) * DC],
                                            fo[:])
                cstart += cw

    nc.compile()
    return nc


def TileCtx(nc):
    return tile.TileContext(nc)


def own_rows(seq, h, IC=None):
    """Rows owned by pair-member h: half of every 128-row block (block-RS)."""
    oh = P // 2
    idx = []
    for k in range(seq // P):
        idx.extend(range(k * P + h * oh, k * P + (h + 1) * oh))
    return np.array(idx)


def make_in_maps(x, W_hidden, b_hidden, W_qk, b_qk, gamma_q, beta_q,
                 gamma_k, beta_k, W_out, b_out, n_cores=8, IC=None):
    """Host-side sharding/layout prep.  Returns per-core input dicts."""
    bf = ml_dtypes.bfloat16
    B, seq, dim = x.shape
    H2 = W_hidden.shape[1]
    H = H2 // 2
    hh = H // 2  # per-core half of v (and of gate)
    nct = hh // P
    in_maps = []
    xT_cache = {}
    for core in range(n_cores):
        b, h = core // 2, core % 2
        if b not in xT_cache:
            xT_cache[b] = np.ascontiguousarray(x[b].T).astype(bf)
        rows = own_rows(seq, h, IC)
        xres = (x[b][rows].astype(np.float32)
                + b_out.astype(np.float32)[None, :])
        cs = slice(h * hh, (h + 1) * hh)
        gs = slice(H + h * hh, H + (h + 1) * hh)
        in_maps.append({
            "xT": xT_cache[b],
            "whv": W_hidden[:, cs].astype(bf),
            "whg": W_hidden[:, gs].astype(bf),
            "wqk": np.ascontiguousarray(
                np.concatenate(np.split(W_qk.astype(bf), dim // P, axis=0),
                               axis=1)),
            "wout": W_out[cs, :].astype(bf),
            "bqk": b_qk.reshape(-1, 1).astype(np.float32),
            "gq": gamma_q.reshape(-1, 1).astype(np.float32),
            "bq": beta_q.reshape(-1, 1).astype(np.float32),
            "gk": gamma_k.reshape(-1, 1).astype(np.float32),
            "bk": beta_k.reshape(-1, 1).astype(np.float32),
            "bhv": b_hidden[cs].reshape(1, -1).astype(bf),
            "bhgT": np.ascontiguousarray(
                b_hidden[gs].reshape(nct, P).T).astype(np.float32),
            "xres": xres,
        })
    return in_maps


_NC_CACHE = {}


def _get_nc(seq, dim, hh, n_cores, with_bhv=True):
    key = (seq, dim, hh, n_cores, with_bhv)
    if key not in _NC_CACHE:
        _NC_CACHE[key] = build_gau_nc(seq=seq, dim=dim, hh=hh,
                                      n_cores=n_cores, with_bhv=with_bhv)
    return _NC_CACHE[key]


def kernel(x, W_hidden, b_hidden, W_qk, b_qk, gamma_q, beta_q, gamma_k,
           beta_k, W_out, b_out):
    x = np.asarray(x)
    B, seq, dim = x.shape
    hh = W_hidden.shape[1] // 4
    n_cores = 2 * B
    with_bhv = bool(np.any(np.asarray(b_hidden)[: 2 * hh] != 0))
    nc = _get_nc(seq, dim, hh, n_cores, with_bhv=with_bhv)
    in_maps = make_in_maps(x, np.asarray(W_hidden), np.asarray(b_hidden),
                           np.asarray(W_qk), np.asarray(b_qk),
                           np.asarray(gamma_q), np.asarray(beta_q),
                           np.asarray(gamma_k), np.asarray(beta_k),
                           np.asarray(W_out), np.asarray(b_out),
                           n_cores=n_cores)
    res = run_bass_kernel_spmd(nc, in_maps, core_ids=list(range(n_cores)))
    out = np.empty((B, seq, dim), np.float32)
    for b in range(B):
        for h in range(2):
            out[b, own_rows(seq, h)] = res.results[2 * b + h]["out"]
    return out



# revision 4
# speedup vs baseline: 1.8980x; 1.8980x over previous
"""GAU (Gated Attention Unit, relu^2 attention) Trainium2 Bass kernel, fp8.

Problem shapes: x [4, 2048, 2048] f32; W_hidden [2048, 8192]; W_qk [2048, 128];
W_out [4096, 2048]; out = GAU(x) + x.

Sharding (8 cores): core = 2*batch + h, h in {0,1}.  Each pair of cores
handles one batch; within the pair the hidden dim is column-split in half
(h picks cols [h*2048:(h+1)*2048] of v and of gate, and rows of W_out).
The qk projection (128 wide) and the 2048x2048 attention matrix are
replicated within the pair.  Each core produces a partial branch output
[2048, 2048] (its W_out-half contribution, bf16); a pairwise per-128-row-
block ReduceScatter(add) writes each core's final [64-row] blocks straight
into the bf16 output tensor.  The host adds the residual x and b_out in
fp32 (the branch is ~0.3% of the output norm, so a bf16 branch costs
~3e-6 relative error).

All five projections run as fp8e4 (e4m3, max +-240) DoubleRow matmuls:
both operands are packed [128, 2, free] pairing two 128-row contraction
tiles, giving ~2 fp8 MACs/PE-cell/cycle.  Quantization scales are powers
of two, chosen so every fp8 tensor's max is >=2.7x below 240 (measured on
the real data distribution; host clips before casting to avoid e4m3's
non-saturating inf at 256):

  x*16, W_hidden*512, W_qk*512, W_out*512, v*16, attn*2^14, og*1024

PSUM stays fp32; descales fold into the activation `scale` ports and DVE
tensor_scalar immediates.  qT/kT and the relu^2 scores matmul stay bf16
(K=128, cheap); the gate is kept resident in SBUF as bf16 pre-scaled by
so/(sa*sv) so the og quantization is a single DVE multiply.  W_out (fp8,
32KB/partition) is resident in SBUF so the output projection can run
i-tile-major, staggering the per-block ReduceScatters behind compute.
"""

import numpy as np
import ml_dtypes
from contextlib import ExitStack

import concourse.bass as bass
import concourse.bacc as bacc
import concourse.mybir as mybir
import concourse.tile as tile
from concourse.bass_utils import run_bass_kernel_spmd

BF16 = mybir.dt.bfloat16
F32 = mybir.dt.float32
F8 = mybir.dt.float8e4
AF = mybir.ActivationFunctionType
ALU = mybir.AluOpType
DR = mybir.MatmulPerfMode.DoubleRow
P = 128
F8MAX = 240.0

# quantization scales (powers of two; see module docstring)
SX = 16.0       # x
SW = 512.0      # W_hidden (both halves), W_qk
SWO = 512.0     # W_out
SV = 16.0       # v activations
SA = 16384.0    # attn = relu(sim)^2    (sqrt(SA) = 128 folds into the relu)
SO = 1024.0     # og activations

INV_H = 1.0 / (SX * SW)        # hidden/qk psum -> real pre-activation
T_V = SV / (SX * SW)           # v-path u multiplier
KG = SO / (SA * SV)            # gate folding so og8 = psum_av * gate_sb
T_G = INV_H * KG               # gate-path u multiplier
RSC = 128.0 / 2048.0           # sqrt(SA)/seq relu scale
INV_O = 1.0 / (SO * SWO)       # out psum -> real partial


def build_gau_nc(seq=2048, dim=2048, hh=2048, n_cores=8, with_bhv=True):
    """Build the SPMD program.  hh = per-core hidden half width."""
    e = P
    nd2 = dim // (2 * P)   # DoubleRow contraction pair tiles
    njt = seq // P
    njp = njt // 2
    nct = hh // P
    ncp = nct // 2
    IC = 512
    n_ic = seq // IC
    CC = 512
    n_cc = hh // CC
    DC = 512
    n_dc = dim // DC
    oh = P // 2
    pairs = [[2 * g, 2 * g + 1] for g in range(n_cores // 2)]

    nc = bacc.Bacc("TRN2", target_bir_lowering=False, debug=False,
                   num_devices=n_cores)

    x8_d = nc.dram_tensor("x8", [P, nd2, 2, seq], F8, kind="ExternalInput")
    whv8_d = nc.dram_tensor("whv8", [P, nd2, 2, hh], F8, kind="ExternalInput")
    whg8_d = nc.dram_tensor("whg8", [P, nd2, 2, hh], F8, kind="ExternalInput")
    wqk8_d = nc.dram_tensor("wqk8", [P, nd2, 2, e], F8, kind="ExternalInput")
    wout8_d = nc.dram_tensor("wout8", [P, ncp, 2, dim], F8,
                             kind="ExternalInput")
    bqk_d = nc.dram_tensor("bqk", [e, 1], F32, kind="ExternalInput")
    bqks_d = nc.dram_tensor("bqks", [e, 1], F32, kind="ExternalInput")
    gq_d = nc.dram_tensor("gq", [e, 1], F32, kind="ExternalInput")
    bq_d = nc.dram_tensor("bq", [e, 1], F32, kind="ExternalInput")
    gk_d = nc.dram_tensor("gk", [e, 1], F32, kind="ExternalInput")
    bk_d = nc.dram_tensor("bk", [e, 1], F32, kind="ExternalInput")
    bhv_d = nc.dram_tensor("bhv", [1, hh], F32, kind="ExternalInput")
    bhgT_d = nc.dram_tensor("bhgT", [P, nct], F32, kind="ExternalInput")
    bhgTs_d = nc.dram_tensor("bhgTs", [P, nct], F32, kind="ExternalInput")
    out_d = nc.dram_tensor("out", [seq // 2, dim], BF16, kind="ExternalOutput")

    with tile.TileContext(nc) as tc, ExitStack() as st:
        constp = st.enter_context(tc.tile_pool(name="const", bufs=1))
        psump = st.enter_context(tc.tile_pool(name="psum", bufs=8, space="PSUM"))
        dramp = st.enter_context(tc.tile_pool(name="dram", bufs=1, space="DRAM"))
        mainp = st.enter_context(tc.tile_pool(name="main", bufs=1))

        # per-128-row-block partial buffers for the pairwise ReduceScatter
        pb = [dramp.tile([P, dim], BF16, tag=f"pb{k}", name=f"pb{k}")
              for k in range(njt)]
        # RS cannot write IO tensors directly; bounce via internal DRAM
        rb = [dramp.tile([P // 2, dim], BF16, tag=f"rb{k}", name=f"rb{k}")
              for k in range(njt)]

        # ---- constants ----
        wqk8_sb = constp.tile([P, nd2, 2, e], F8, tag="wqk8")
        nc.sync.dma_start(wqk8_sb[:], wqk8_d[:])
        bqk_sb = constp.tile([e, 1], F32, tag="bqk")
        nc.sync.dma_start(bqk_sb[:], bqk_d[:])
        bqks_sb = constp.tile([e, 1], F32, tag="bqks")
        nc.sync.dma_start(bqks_sb[:], bqks_d[:])
        gq_sb = constp.tile([e, 1], F32, tag="gq")
        nc.sync.dma_start(gq_sb[:], gq_d[:])
        bq_sb = constp.tile([e, 1], F32, tag="bq")
        nc.sync.dma_start(bq_sb[:], bq_d[:])
        gk_sb = constp.tile([e, 1], F32, tag="gk")
        nc.sync.dma_start(gk_sb[:], gk_d[:])
        bk_sb = constp.tile([e, 1], F32, tag="bk")
        nc.sync.dma_start(bk_sb[:], bk_d[:])
        bhg_sb = constp.tile([P, nct], F32, tag="bhg")
        nc.sync.dma_start(bhg_sb[:], bhgT_d[:])
        bhgs_sb = constp.tile([P, nct], F32, tag="bhgs")
        nc.sync.dma_start(bhgs_sb[:], bhgTs_d[:])
        bhv_sb = constp.tile([1, hh], F32, tag="bhv")
        nc.sync.dma_start(bhv_sb[:], bhv_d[:])
        ones_sb = constp.tile([1, P], F32, tag="ones")
        nc.vector.memset(ones_sb[:], 1.0)

        # tiny ReduceScatter to warm the collective stream (first real RS
        # otherwise pays a ~50us cold-start)
        warm_in = dramp.tile([2, 64], F32, tag="warm_in", name="warm_in")
        warm_out = dramp.tile([1, 64], F32, tag="warm_out", name="warm_out")
        warm_sb = constp.tile([2, 64], F32, tag="warm_sb")
        nc.vector.memset(warm_sb[:], 0.0)
        nc.gpsimd.dma_start(warm_in[:], warm_sb[:])
        nc.gpsimd.collective_compute("ReduceScatter", ALU.add,
                                     replica_groups=pairs,
                                     ins=[warm_in.opt()],
                                     outs=[warm_out.opt()])

        # persistent activations
        qT_sb = mainp.tile([e, seq], BF16, tag="qT", name="qT")
        kT_sb = mainp.tile([e, seq], BF16, tag="kT", name="kT")
        v8_sb = mainp.tile([P, njt, hh], F8, tag="v8", name="v8")
        gate_sb = mainp.tile([P, nct, seq], BF16, tag="gate", name="gate")

        with tc.tile_pool(name="ph1", bufs=1) as ph1p, \
             tc.tile_pool(name="wstream", bufs=1) as wsp:
            x8_sb = ph1p.tile([P, nd2, 2, seq], F8, tag="x8", name="x8")
            for half in range(2):
                cols = slice(half * (seq // 2), (half + 1) * (seq // 2))
                nc.sync.dma_start(x8_sb[:, :, :, cols], x8_d[:, :, :, cols])

            # ---- qk projection (fp8 DoubleRow) ----
            # silu(u) = u * sigmoid(u); no Silu LUT in the interp, so build
            # from Sigmoid (ACT) + mult (DVE).
            with tc.tile_pool(name="qkp", bufs=1) as qkp:
                qk_sb = qkp.tile([e, seq], F32, tag="qk", name="qk")
                for ic in range(n_ic):
                    cols = slice(ic * IC, (ic + 1) * IC)
                    ps = psump.tile([e, IC], F32, tag="ps", name="ps")
                    for dp in range(nd2):
                        nc.tensor.matmul(ps[:], wqk8_sb[:, dp],
                                         x8_sb[:, dp, :, cols],
                                         start=(dp == 0), stop=(dp == nd2 - 1),
                                         perf_mode=DR)
                    sg = qkp.tile([e, IC], F32, tag="sg1", bufs=2, name="sg")
                    nc.scalar.activation(sg[:], ps[:], AF.Sigmoid,
                                         bias=bqk_sb[:], scale=INV_H)
                    u = qkp.tile([e, IC], F32, tag="u1", bufs=2, name="u")
                    nc.vector.tensor_scalar(u[:], ps[:], bqks_sb[:], INV_H,
                                            ALU.add, ALU.mult)
                    nc.vector.tensor_tensor(qk_sb[:, cols], u[:], sg[:],
                                            ALU.mult)
                nc.vector.tensor_scalar(qT_sb[:], qk_sb[:], gq_sb[:],
                                        bq_sb[:], ALU.mult, ALU.add)
                nc.vector.tensor_scalar(kT_sb[:], qk_sb[:], gk_sb[:],
                                        bk_sb[:], ALU.mult, ALU.add)

            # ---- hidden, v part: v[j, c] (fp8 DoubleRow) ----
            for cc in range(n_cc):
                ccs = slice(cc * CC, (cc + 1) * CC)
                wv8 = [wsp.tile([P, 2, CC], F8, tag=f"wv{dp}", bufs=2,
                                name=f"wv{dp}") for dp in range(nd2)]
                for dp in range(nd2):
                    nc.sync.dma_start(wv8[dp][:], whv8_d[:, dp, :, ccs])
                for jt in range(njt):
                    jts = slice(jt * P, (jt + 1) * P)
                    ps = psump.tile([P, CC], F32, tag="ps", name="ps")
                    for dp in range(nd2):
                        nc.tensor.matmul(ps[:], x8_sb[:, dp, :, jts],
                                         wv8[dp][:], start=(dp == 0),
                                         stop=(not with_bhv and dp == nd2 - 1),
                                         perf_mode=DR)
                    if with_bhv:
                        nc.tensor.matmul(ps[:], ones_sb[:], bhv_sb[:, ccs],
                                         start=False, stop=True)
                    sg = wsp.tile([P, CC], F32, tag="sgv", bufs=2, name="sgv")
                    nc.scalar.activation(sg[:], ps[:], AF.Sigmoid, scale=INV_H)
                    t = wsp.tile([P, CC], F32, tag="tv", bufs=2, name="tv")
                    nc.vector.tensor_scalar(t[:], ps[:], T_V, None, ALU.mult)
                    nc.vector.tensor_tensor(v8_sb[:, jt, ccs], t[:], sg[:],
                                            ALU.mult)

            # ---- hidden, gate part: gateT[c, i], resident in SBUF ----
            # stored pre-scaled by KG = so/(sa*sv) so the phase-2 og
            # quantization is a single DVE multiply against the attn@v psum
            for ct in range(nct):
                cts = slice(ct * P, (ct + 1) * P)
                wg8 = [wsp.tile([P, 2, P], F8, tag=f"wg{dp}", bufs=2,
                                name=f"wg{dp}") for dp in range(nd2)]
                for dp in range(nd2):
                    nc.sync.dma_start(wg8[dp][:], whg8_d[:, dp, :, cts])
                for ic in range(n_ic):
                    cols = slice(ic * IC, (ic + 1) * IC)
                    ps = psump.tile([P, IC], F32, tag="ps", name="ps")
                    for dp in range(nd2):
                        nc.tensor.matmul(ps[:], wg8[dp][:],
                                         x8_sb[:, dp, :, cols],
                                         start=(dp == 0), stop=(dp == nd2 - 1),
                                         perf_mode=DR)
                    sg = wsp.tile([P, IC], F32, tag="sgg", bufs=2, name="sgg")
                    nc.scalar.activation(sg[:], ps[:], AF.Sigmoid,
                                         bias=bhg_sb[:, ct:ct + 1],
                                         scale=INV_H)
                    t = wsp.tile([P, IC], F32, tag="tg", bufs=2, name="tg")
                    nc.vector.tensor_scalar(t[:], ps[:],
                                            bhgs_sb[:, ct:ct + 1], T_G,
                                            ALU.add, ALU.mult)
                    nc.vector.tensor_tensor(gate_sb[:, ct, cols], t[:], sg[:],
                                            ALU.mult)

        # ---- attention + output, per i-chunk ----
        with tc.tile_pool(name="ph2", bufs=1) as ph2p:
            wout8_sb = ph2p.tile([P, ncp, 2, dim], F8, tag="wout8",
                                 name="wout8")
            nc.sync.dma_start(wout8_sb[:], wout8_d[:])
            og8_sb = ph2p.tile([P, nct, IC], F8, tag="og8", name="og8")
            for c in range(n_ic):
                chunk = slice(c * IC, (c + 1) * IC)
                # attnT[j, chunk] = relu(sim*sqrt(sa)/seq)^2 -> fp8
                at8 = ph2p.tile([P, njt, IC], F8, tag="at8", bufs=2,
                                name="at8")
                for jt in range(njt):
                    ps = psump.tile([P, IC], F32, tag="ps", name="ps")
                    nc.tensor.matmul(ps[:], kT_sb[:, jt * P:(jt + 1) * P],
                                     qT_sb[:, chunk], start=True, stop=True)
                    rstage = ph2p.tile([P, IC], BF16, tag="rstage", bufs=4,
                                       name="rstage")
                    nc.scalar.activation(rstage[:], ps[:], AF.Relu, scale=RSC)
                    nc.vector.tensor_tensor(at8[:, jt, :], rstage[:],
                                            rstage[:], ALU.mult)
                # ogT[c, chunk] = (v^T @ attnT) * gate  (fp8 DoubleRow)
                for ct in range(nct):
                    cts = slice(ct * P, (ct + 1) * P)
                    ps = psump.tile([P, IC], F32, tag="ps", name="ps")
                    for jp in range(njp):
                        nc.tensor.matmul(ps[:], v8_sb[:, 2 * jp:2 * jp + 2, cts],
                                         at8[:, 2 * jp:2 * jp + 2, :],
                                         start=(jp == 0), stop=(jp == njp - 1),
                                         perf_mode=DR)
                    nc.vector.tensor_tensor(og8_sb[:, ct, :], ps[:],
                                            gate_sb[:, ct, chunk], ALU.mult)
                # partial[i-tile rows, :] = ogT^T @ Wout; i-tile-major so each
                # block's pairwise ReduceScatter launches as soon as its rows
                # are done and overlaps the remaining compute
                for it in range(IC // P):
                    k = c * (IC // P) + it
                    its = slice(it * P, (it + 1) * P)
                    for dc in range(n_dc):
                        dcs = slice(dc * DC, (dc + 1) * DC)
                        ps = psump.tile([P, DC], F32, tag="ps", name="ps")
                        for cp in range(ncp):
                            nc.tensor.matmul(ps[:],
                                             og8_sb[:, 2 * cp:2 * cp + 2, its],
                                             wout8_sb[:, cp, :, dcs],
                                             start=(cp == 0),
                                             stop=(cp == ncp - 1),
                                             perf_mode=DR)
                        po = ph2p.tile([P, DC], BF16, tag="po", bufs=4,
                                       name="po")
                        nc.vector.tensor_scalar(po[:], ps[:], INV_O, None,
                                                ALU.mult)
                        nc.scalar.dma_start(pb[k][:, dcs], po[:])
                    nc.gpsimd.collective_compute(
                        "ReduceScatter", ALU.add, replica_groups=pairs,
                        ins=[pb[k].opt()],
                        outs=[rb[k].opt()])
                    nc.gpsimd.dma_start(out_d[k * oh:(k + 1) * oh, :],
                                        rb[k][:])

    nc.compile()
    return nc


def own_rows(seq, h):
    """Rows owned by pair-member h: half of every 128-row block."""
    oh = P // 2
    idx = []
    for k in range(seq // P):
        idx.extend(range(k * P + h * oh, k * P + (h + 1) * oh))
    return np.array(idx)


def _q8(a, s):
    """Quantize a*s to fp8 e4m3, clipping to +-240 (TRN e4m3 infs at 256)."""
    return np.clip(a.astype(np.float32) * s, -F8MAX, F8MAX).astype(
        ml_dtypes.float8_e4m3)


def _dr_pack(m8):
    """[K, F] fp8 -> [128, K//256, 2, F] DoubleRow pair layout."""
    K, F = m8.shape
    return np.ascontiguousarray(
        m8.reshape(K // 256, 2, P, F).transpose(2, 0, 1, 3))


def make_in_maps(x, W_hidden, b_hidden, W_qk, b_qk, gamma_q, beta_q,
                 gamma_k, beta_k, W_out, b_out, n_cores=8):
    """Host-side quantization/layout prep.  Returns per-core input dicts."""
    B, seq, dim = x.shape
    H2 = W_hidden.shape[1]
    H = H2 // 2
    hh = H // 2
    nct = hh // P
    f32 = np.float32
    x8_cache = {}
    half_cache = {}

    def halves(h):
        if h not in half_cache:
            cs = slice(h * hh, (h + 1) * hh)
            gs = slice(H + h * hh, H + (h + 1) * hh)
            half_cache[h] = {
                "whv8": _dr_pack(_q8(W_hidden[:, cs], SW)),
                "whg8": _dr_pack(_q8(W_hidden[:, gs], SW)),
                "wout8": _dr_pack(_q8(W_out[cs, :], SWO)),
                "bhv": (b_hidden[cs].astype(f32) * (SX * SW)).reshape(1, -1),
                "bhgT": np.ascontiguousarray(
                    b_hidden[gs].astype(f32).reshape(nct, P).T),
            }
        return half_cache[h]

    wqk8 = _dr_pack(_q8(W_qk, SW))
    in_maps = []
    for core in range(n_cores):
        b, h = core // 2, core % 2
        if b not in x8_cache:
            x8_cache[b] = _dr_pack(_q8(np.ascontiguousarray(x[b].T), SX))
        hv = halves(h)
        in_maps.append({
            "x8": x8_cache[b],
            "whv8": hv["whv8"],
            "whg8": hv["whg8"],
            "wout8": hv["wout8"],
            "wqk8": wqk8,
            "bqk": b_qk.reshape(-1, 1).astype(f32),
            "bqks": (b_qk.reshape(-1, 1).astype(f32) * (SX * SW)),
            "gq": gamma_q.reshape(-1, 1).astype(f32),
            "bq": beta_q.reshape(-1, 1).astype(f32),
            "gk": gamma_k.reshape(-1, 1).astype(f32),
            "bk": beta_k.reshape(-1, 1).astype(f32),
            "bhv": hv["bhv"],
            "bhgT": hv["bhgT"],
            "bhgTs": hv["bhgT"] * (SX * SW),
        })
    return in_maps


_NC_CACHE = {}


def _get_nc(seq, dim, hh, n_cores, with_bhv=True):
    key = (seq, dim, hh, n_cores, with_bhv)
    if key not in _NC_CACHE:
        _NC_CACHE[key] = build_gau_nc(seq=seq, dim=dim, hh=hh,
                                      n_cores=n_cores, with_bhv=with_bhv)
    return _NC_CACHE[key]


def kernel(x, W_hidden, b_hidden, W_qk, b_qk, gamma_q, beta_q, gamma_k,
           beta_k, W_out, b_out):
    x = np.asarray(x)
    B, seq, dim = x.shape
    hh = W_hidden.shape[1] // 4
    n_cores = 2 * B
    with_bhv = bool(np.any(np.asarray(b_hidden)[: 2 * hh] != 0))
    nc = _get_nc(seq, dim, hh, n_cores, with_bhv=with_bhv)
    in_maps = make_in_maps(x, np.asarray(W_hidden), np.asarray(b_hidden),
                           np.asarray(W_qk), np.asarray(b_qk),
                           np.asarray(gamma_q), np.asarray(beta_q),
                           np.asarray(gamma_k), np.asarray(beta_k),
                           np.asarray(W_out), np.asarray(b_out),
                           n_cores=n_cores)
    res = run_bass_kernel_spmd(nc, in_maps, core_ids=list(range(n_cores)))
    bo = np.asarray(b_out).astype(np.float32)
    out = np.empty((B, seq, dim), np.float32)
    for b in range(B):
        for h in range(2):
            rows = own_rows(seq, h)
            out[b, rows] = (res.results[2 * b + h]["out"].astype(np.float32)
                            + x[b][rows] + bo)
    return out
